# revision 18
# baseline (speedup 1.0000x reference)
"""DGCN layer (message passing GNN) on 8 Trainium2 NeuronCores via Bass/Tile.

Strategy (matches the dst-sharded hint):
  - Nodes are range-partitioned across the 8 cores (6250 nodes/core).
  - Each core owns every edge whose dst lies in its node range, so the
    segment-sum over dst is fully core-local.
  - h (pre-scaled by outdeg^-0.5, cast to bf16) is replicated into each
    core's HBM at input staging time (this plays the role of the all-gather
    of src features); the per-edge random feat[src] read is an on-device
    dma_gather (custom SWDGE ucode) spread over 4 SWDGE queues.
  - The guaranteed self-loop edges (first N edges, src==dst) never touch
    the gather: their contribution alpha^d_v * feat[v] @ W is added in
    phase 2 as a second matmul (lhsT = host-staged, alpha-scaled own
    features transposed) accumulating into the same PSUM group.
  - Nodes are degree-ranked into windows (rank block w -> window w on all
    cores), so per-window tile counts are shared across cores (SPMD-safe)
    yet track the actual edge counts: early windows are tall, late windows
    short, with ~no padding. Within a window, each core's lo/hi count is
    padded to the 8-core max with index-0 entries so num_idxs_reg (the
    exact non-negative index count the ucode contract requires) is a
    shared compile-time constant; -1 entries beyond it generate no
    descriptors at all.
  - dma_gather indices are int16 (< 32768), so h is addressed as two
    tables (rows [0, 32768) and [32768, N)).
  - Windows are 64 dst nodes wide: the DVE sel-matrix build cost scales
    with window width, while the PE cost is LdWeights-bound and doesn't.

Device pipeline per core (all phase-1 operands bf16, PSUM fp32):
  phase 1 (edge aggregation, accumulates agg^T[feat, node] in SBUF):
    per 64-dst-node window: dma_gathers (lo/hi tables) fetch the window's
    h[src] rows; sel tiles are built up to 16-at-a-time with two wide DVE
    ops (is_equal against iota, then mult by alpha^dist), psum += matmul(
    lhsT=G_tile, rhs=sel_tile) accumulating over the window; the finished
    window column-block is copied (cast to bf16) into agg^T.
  phase 2: per 128-node block (2 windows): psum = matmul(lhsT=hsT block,
    rhs=W) + matmul(lhsT=agg^T block, rhs=W), then * s_v (per-partition,
    fp32) + bias into a staging SBUF tile; results leave in two large
    partition-major DMAs (mid-stream and final).
"""

import math

import numpy as np

P = 128
WIN = 64  # dst-window width (nodes per psum accumulation group)
ALPHA = 0.5
N_CORES = 8
SPLIT = 32768  # int16 index limit for dma_gather
KW = 16  # max sel tiles built per wide DVE op
GCH = 8  # max tiles per dma_gather (hw limit: <=1024 idxs/inst)


def _wrap_idx16(flat):
    """dma_gather index layout: entry k -> partition k%16, column k//16,
    replicated across the 8 gpsimd core groups (partitions 16-127)."""
    n = flat.shape[-1]
    assert n % 16 == 0
    cols = n // 16
    w = np.asarray(flat, np.int16).reshape(cols, 16).T  # [16, cols]
    return np.tile(w, (8, 1))  # [128, cols]


def _prep_host(src, dst, distance, n_cores, N):
    """Shard edges by dst; build per-core padded tile arrays.

    Returns per-window tile/index-count lists shared by all cores, plus the
    per-core gather/sel constants and the output inverse permutation.
    """
    E = src.shape[0]

    src = np.asarray(src).astype(np.int64)
    dst = np.asarray(dst).astype(np.int64)
    distance = np.asarray(distance).astype(np.int64)

    in_deg = np.bincount(dst, minlength=N).astype(np.float64)
    s_all = in_deg**-1.5  # applied after the W matmul

    # structural self-loops (first N edges): handled algebraically in
    # phase 2; excluded from the gather stream
    eidx = np.arange(E)
    self_mask = (eidx < N) & (src == dst)
    d_self = np.full(N, -1, np.int64)
    d_self[src[self_mask]] = distance[self_mask]

    gsrc = src[~self_mask]
    gdst = dst[~self_mask]
    gdist = distance[~self_mask]
    coef_all = np.float64(ALPHA) ** gdist.astype(np.float64)

    # Degree-ranked node -> (core, window, slot): rank block w (512 nodes)
    # becomes window w on every core; within a block, nodes are dealt
    # round-robin over cores in hi-degree order so the 8 per-core counts
    # stay within a few edges of each other.
    lo_deg = np.bincount(gdst[gsrc < SPLIT], minlength=N).astype(np.int64)
    hi_deg = np.bincount(gdst[gsrc >= SPLIT], minlength=N).astype(np.int64)
    order_nodes = np.argsort(-(lo_deg + hi_deg), kind="stable")
    blk = n_cores * WIN
    n_windows = (N + blk - 1) // blk
    node_core = np.empty(N, np.int64)
    node_window = np.empty(N, np.int64)
    node_slot = np.empty(N, np.int64)
    for w in range(n_windows):
        nodes_b = order_nodes[w * blk : (w + 1) * blk]
        nodes_b = nodes_b[np.argsort(-hi_deg[nodes_b], kind="stable")]
        pos = np.arange(nodes_b.shape[0])
        node_core[nodes_b] = pos % n_cores
        node_window[nodes_b] = w
        node_slot[nodes_b] = pos // n_cores

    core_of = node_core[gdst]
    w_of = node_window[gdst]
    r_of = node_slot[gdst].astype(np.float32)
    is_hi = (gsrc >= SPLIT).astype(np.int64)

    # per-(core, window, lo/hi) counts -> shared per-window capacities
    gw = (core_of * n_windows + w_of) * 2 + is_hi
    counts = np.bincount(gw, minlength=n_cores * n_windows * 2).reshape(
        n_cores, n_windows, 2
    )
    maxc = counts.max(axis=0)  # [n_windows, 2]
    reg_lo = (maxc[:, 0] + 15) // 16 * 16  # exact shared non-neg idx count
    reg_hi = (maxc[:, 1] + 15) // 16 * 16
    t_lo = np.maximum((reg_lo + P - 1) // P, 0)
    t_hi = np.maximum((reg_hi + P - 1) // P, 0)
    t_tot = t_lo + t_hi
    col_off = np.concatenate([[0], np.cumsum(t_tot)[:-1]])
    n_cols = int(t_tot.sum())

    # order edges by (core, window, lo/hi); q = position within group
    order = np.argsort(gw, kind="stable")
    sgw = gw[order]
    flat_counts = counts.reshape(-1)
    win_start = np.concatenate([[0], np.cumsum(flat_counts)[:-1]])
    q = np.arange(sgw.shape[0], dtype=np.int64) - win_start[sgw]

    core_arr = sgw // (2 * n_windows)
    w_arr = (sgw // 2) % n_windows
    hi_arr = sgw % 2
    j_arr = q // P + hi_arr * t_lo[w_arr]  # hi tiles after lo tiles
    p_arr = q % P
    col_arr = col_off[w_arr] + j_arr

    # r = -1 on pad slots (is_equal against iota 0..63 is then always 0)
    rofs = np.full((n_cores, P, n_cols), -1.0, np.float32)
    coef = np.zeros((n_cores, P, n_cols), np.float32)
    rofs[core_arr, p_arr, col_arr] = r_of[order]
    coef[core_arr, p_arr, col_arr] = coef_all[order].astype(np.float32)

    # gather indices, table-relative; -1 marks no-descriptor entries
    srcrel = np.full((n_cores, P, n_cols), -1, np.int64)
    srcrel[core_arr, p_arr, col_arr] = gsrc[order] - (gsrc[order] >= SPLIT) * SPLIT

    # pad every core's (window, lo/hi) group up to the shared reg count
    # with index 0 (a real descriptor; nullified by sel) so the shared
    # num_idxs_reg matches each core's non-negative count exactly
    for c in range(n_cores):
        for w in range(n_windows):
            for h2, reg in ((0, reg_lo[w]), (1, reg_hi[w])):
                cnt = counts[c, w, h2]
                j0 = col_off[w] + (t_lo[w] if h2 else 0)
                # flat positions [cnt, reg) within this group get idx 0
                if cnt < reg:
                    ppos = np.arange(cnt, reg)
                    srcrel[c, ppos % P, j0 + ppos // P] = 0

    # wrapped idx16 with per-window variable block sizes, 64B-aligned
    CLa = (t_lo * 8 + 31) // 32 * 32
    CHa = (t_hi * 8 + 31) // 32 * 32
    idx_off = np.concatenate([[0], np.cumsum(CLa + CHa)[:-1]])
    idx_cols = int((CLa + CHa).sum())
    idx16 = np.full((n_cores, P, idx_cols), -1, np.int16)
    for c in range(n_cores):
        flat = srcrel[c].T  # [n_cols, P]: (tile, lane)
        for w in range(n_windows):
            j0 = col_off[w]
            lo = flat[j0 : j0 + t_lo[w]].reshape(-1)
            hi = flat[j0 + t_lo[w] : j0 + t_tot[w]].reshape(-1)
            base = idx_off[w]
            if t_lo[w]:
                idx16[c, :, base : base + t_lo[w] * 8] = _wrap_idx16(lo)
            if t_hi[w]:
                idx16[c, :, base + CLa[w] : base + CLa[w] + t_hi[w] * 8] = (
                    _wrap_idx16(hi)
                )

    # per-node output scale, laid out for phase-2 blocks of 128 rows
    n_blocks = n_windows * WIN // P
    snode = np.ones((n_cores, P, n_blocks), np.float32)
    out_row = node_window * WIN + node_slot
    snode[node_core, out_row % P, out_row // P] = s_all.astype(np.float32)

    return (
        idx16, rofs, coef, snode, node_core, out_row, d_self,
        n_windows, t_lo, t_hi, reg_lo, reg_hi, col_off, CLa, CHa, idx_off,
        n_cols, idx_cols,
    )


def _build_nc(N, D, plan):
    import concourse.bacc as bacc
    import concourse.tile as tile
    from concourse import mybir

    (
        n_windows, t_lo, t_hi, reg_lo, reg_hi, col_off, CLa, CHa, idx_off,
        n_cols, idx_cols,
    ) = plan

    f32 = mybir.dt.float32
    bf16 = mybir.dt.bfloat16
    i16 = mybir.dt.int16
    n_blocks = n_windows * WIN // P  # phase-2 blocks of 128 output rows

    # bconst (bf16) free-dim layout: rofs | coef | iota | wmat
    btot = 2 * n_cols + WIN + D
    # fconst (f32) free-dim layout: biasf | snode
    ftot = D + n_blocks

    nc = bacc.Bacc(
        None,
        target_bir_lowering=False,
        debug=False,
        num_swdge_queues=2,
        dynamic_dma_scratch_size=65536,
    )
    h_d = nc.declare_dram_parameter("h", [N, D], bf16, isOutput=False)
    idx_d = nc.declare_dram_parameter("idx16", [P, idx_cols], i16, isOutput=False)
    bc_d = nc.declare_dram_parameter("bconst", [P, btot], bf16, isOutput=False)
    fc_d = nc.declare_dram_parameter("fconst", [P, ftot], f32, isOutput=False)
    # self-loop contribution: alpha^d_v * feat[v], transposed, block layout
    hs_d = nc.declare_dram_parameter("hself", [P, n_blocks * D], bf16, isOutput=False)
    # partition-major output: out[p, k*D:(k+1)*D] = block k, row p
    out_d = nc.declare_dram_parameter("out", [P, n_blocks * D], f32, isOutput=True)

    mult = mybir.AluOpType.mult

    with tile.TileContext(nc) as tc:
        with (
            tc.tile_pool(name="singles", bufs=1) as singles,
            tc.tile_pool(name="glo", bufs=4) as glopool,
            tc.tile_pool(name="ghi", bufs=4) as ghipool,
            tc.tile_pool(name="sel", bufs=4) as selpool,
            tc.tile_pool(name="psum", bufs=4, space="PSUM") as psumpool,
            tc.tile_pool(name="psum2", bufs=2, space="PSUM") as psum2pool,
        ):
            # idx chunks first on the sync HWDGE queue (they gate the first
            # gathers); big constants ride the Activation HWDGE queue so
            # they don't delay them
            idx_sb = singles.tile([P, idx_cols], i16)
            hd = min(idx_cols, max(256, int(idx_off[min(4, n_windows - 1)])))
            nc.sync.dma_start(out=idx_sb[:, :hd], in_=idx_d[:, :hd])
            if hd < idx_cols:
                nc.sync.dma_start(out=idx_sb[:, hd:], in_=idx_d[:, hd:])
            bc_sb = singles.tile([P, btot], bf16)
            nc.scalar.dma_start(out=bc_sb[:], in_=bc_d[:])
            fc_sb = singles.tile([P, ftot], f32)
            nc.scalar.dma_start(out=fc_sb[:], in_=fc_d[:])
            hs_sb = singles.tile([P, n_blocks * D], bf16)
            nc.scalar.dma_start(out=hs_sb[:], in_=hs_d[:])

            r_sb = bc_sb[:, 0:n_cols]
            c_sb = bc_sb[:, n_cols : 2 * n_cols]
            o0 = 2 * n_cols
            io_sb = bc_sb[:, o0 : o0 + WIN]
            w_sb = bc_sb[:, o0 + WIN : o0 + WIN + D]
            b_sb = fc_sb[:, 0:D]
            s_sb = fc_sb[:, D : D + n_blocks]

            agg = singles.tile([P, n_windows * WIN], bf16)  # agg^T [feat, node]
            o_all = singles.tile([P, n_blocks * D], f32)  # staged outputs

            def _phase2(k):
                ps2 = psum2pool.tile([P, D], f32)
                nc.tensor.matmul(
                    out=ps2[:],
                    lhsT=hs_sb[:, k * P : (k + 1) * P],
                    rhs=w_sb,
                    start=True,
                    stop=False,
                )
                nc.tensor.matmul(
                    out=ps2[:],
                    lhsT=agg[:, k * P : (k + 1) * P],
                    rhs=w_sb,
                    start=False,
                    stop=True,
                )
                o = o_all[:, k * D : (k + 1) * D]
                nc.vector.tensor_tensor(
                    out=o,
                    in0=ps2[:],
                    in1=s_sb[:, k : k + 1].to_broadcast([P, D]),
                    op=mult,
                )
                nc.vector.tensor_add(out=o, in0=o, in1=b_sb)

            h_lo = h_d[0 : min(SPLIT, N), :]
            hi_base = SPLIT if N > SPLIT else 0
            h_hi = h_d[hi_base:N, :]

            # Zero every physical gather buffer once: entries beyond
            # num_idxs_reg generate no descriptor, so whatever the buffer
            # holds leaks into the matmul lhsT. After this, stale contents
            # are always finite bf16 (earlier gathered rows), which sel's 0
            # column nullifies; raw uninitialized SBUF could decode as
            # NaN/Inf and NaN*0 would poison the accumulation.
            for _ in range(4):
                gz = glopool.tile([P, GCH, P], bf16, tag="glo")
                nc.vector.memset(gz[:], 0.0)
                gz = ghipool.tile([P, GCH, P], bf16, tag="ghi")
                nc.vector.memset(gz[:], 0.0)

            # phase-2 burst points (even window counts): draining output
            # work mid-stream keeps the tail after the last gather short
            mid1 = (n_windows // 2) // 2 * 2
            mid2 = (11 * n_windows // 12) // 2 * 2
            qctr = 0
            for w in range(n_windows):
                T_lo_w, T_hi_w = int(t_lo[w]), int(t_hi[w])
                T_w = T_lo_w + T_hi_w
                if T_w == 0:
                    nc.vector.memset(agg[:, w * WIN : (w + 1) * WIN], 0.0)
                    continue
                base = int(idx_off[w])
                chunks = []  # (tile_buf, ntiles) in tile order lo then hi
                for h2, T_t, reg, tbl, cb0, pool, tg in (
                    (0, T_lo_w, int(reg_lo[w]), h_lo, base, glopool, "glo"),
                    (1, T_hi_w, int(reg_hi[w]), h_hi, base + int(CLa[w]), ghipool, "ghi"),
                ):
                    for k in range((T_t + GCH - 1) // GCH):
                        nt = min(GCH, T_t - k * GCH)
                        creg = min(max(reg - k * GCH * P, 0), nt * P)
                        g = pool.tile([P, GCH, P], bf16, tag=tg)
                        cb = cb0 + k * GCH * 8
                        qn = qctr % 2
                        qctr += 1
                        nc.gpsimd.dma_gather(
                            g[:, :nt, :],
                            tbl,
                            idx_sb[:, cb : cb + nt * 8],
                            nt * P,
                            creg,
                            P,
                            single_packet=False,
                            queue_num=qn,
                        )
                        chunks.append((g, nt))

                # sel tiles for the whole window, built KW at a time with
                # two wide DVE ops (bf16 2x mode)
                sel_tiles = []
                for k0 in range(0, T_w, KW):
                    kk = min(KW, T_w - k0)
                    sw = selpool.tile([P, KW, WIN], bf16, tag="sel")
                    t0 = int(col_off[w]) + k0
                    r3 = r_sb[:, t0 : t0 + kk, None].to_broadcast([P, kk, WIN])
                    c3 = c_sb[:, t0 : t0 + kk, None].to_broadcast([P, kk, WIN])
                    io3 = io_sb[:, None, :].to_broadcast([P, kk, WIN])
                    nc.vector.tensor_tensor(
                        out=sw[:, :kk, :],
                        in0=r3,
                        in1=io3,
                        op=mybir.AluOpType.is_equal,
                    )
                    nc.vector.tensor_tensor(
                        out=sw[:, :kk, :], in0=sw[:, :kk, :], in1=c3, op=mult
                    )
                    sel_tiles.append(sw)

                ps = psumpool.tile([P, WIN], f32)
                j = 0
                for g, nt in chunks:
                    for jj in range(nt):
                        nc.tensor.matmul(
                            out=ps[:],
                            lhsT=g[:, jj, :],
                            rhs=sel_tiles[j // KW][:, j % KW, :],
                            start=(j == 0),
                            stop=(j == T_w - 1),
                        )
                        j += 1
                nc.scalar.copy(out=agg[:, w * WIN : (w + 1) * WIN], in_=ps[:])

                if w in (mid1 - 1, mid2 - 1):
                    # mid-stream burst: finish output for the windows already
                    # aggregated, while gathers for the rest continue
                    k0 = 0 if w == mid1 - 1 else mid1 * WIN // P
                    k1 = (w + 1) * WIN // P
                    for k in range(k0, k1):
                        _phase2(k)
                    nc.scalar.dma_start(
                        out=out_d[:, k0 * D : k1 * D],
                        in_=o_all[:, k0 * D : k1 * D],
                    )
            k2 = mid2 * WIN // P
            for k in range(k2, n_blocks):
                _phase2(k)
            nc.scalar.dma_start(out=out_d[:, k2 * D :], in_=o_all[:, k2 * D :])

    nc.compile()
    return nc


def kernel(h, src, dst, distance, weight, bias, _trace=False):
    import ml_dtypes
    from concourse.bass_utils import run_bass_kernel_spmd

    h = np.asarray(h, dtype=np.float32)
    weight = np.asarray(weight, dtype=np.float32)
    bias = np.asarray(bias, dtype=np.float32)
    N, D = h.shape

    (
        idx16, rofs, coef, snode, node_core, out_row, d_self,
        n_windows, t_lo, t_hi, reg_lo, reg_hi, col_off, CLa, CHa, idx_off,
        n_cols, idx_cols,
    ) = _prep_host(src, dst, distance, N_CORES, N)

    # source-side norm folded into the (bf16) feature table
    out_deg = np.bincount(np.asarray(src).astype(np.int64), minlength=N)
    h_pre = h * (out_deg.astype(np.float64) ** -0.5)[:, None].astype(np.float32)
    h_bf = np.ascontiguousarray(h_pre.astype(ml_dtypes.bfloat16))

    # self-loop table: hsT[:, out_row[v]] = alpha^d_self[v] * feat[v]
    n_blocks = n_windows * WIN // P
    hsT = np.zeros((N_CORES, D, n_blocks * P), np.float32)
    has_self = d_self >= 0
    vs = np.nonzero(has_self)[0]
    hsT[node_core[vs], :, out_row[vs]] = (
        h_pre[vs] * (np.float32(ALPHA) ** d_self[vs].astype(np.float32))[:, None]
    )

    iota = np.broadcast_to(np.arange(WIN, dtype=np.float32)[None, :], (P, WIN))
    biasf = np.broadcast_to(bias[None, :], (P, D))

    plan = (
        n_windows, t_lo, t_hi, reg_lo, reg_hi, col_off, CLa, CHa, idx_off,
        n_cols, idx_cols,
    )
    nc = _build_nc(N, D, plan)

    in_maps = []
    for c in range(N_CORES):
        bconst = np.concatenate([rofs[c], coef[c], iota, weight], axis=1).astype(
            ml_dtypes.bfloat16
        )
        fconst = np.concatenate([biasf, snode[c]], axis=1).astype(np.float32)
        in_maps.append(
            {
                "h": h_bf,
                "idx16": np.ascontiguousarray(idx16[c]),
                "bconst": np.ascontiguousarray(bconst),
                "fconst": np.ascontiguousarray(fconst),
                "hself": np.ascontiguousarray(hsT[c].astype(ml_dtypes.bfloat16)),
            }
        )

    res = run_bass_kernel_spmd(nc, in_maps, list(range(N_CORES)), trace=_trace)

    # out[p, k*D:(k+1)*D] holds node (core, row k*128+p); un-permute
    stacked = np.stack([res.results[c]["out"] for c in range(N_CORES)])
    per_core = stacked.reshape(N_CORES, P, n_blocks, D).transpose(0, 2, 1, 3)
    out = per_core[node_core, out_row // P, out_row % P].astype(np.float32)

    if _trace:
        return out, res
    return out


# revision 19
# speedup vs baseline: 2.0531x; 2.0531x over previous
"""DGCN layer (message passing GNN) on 8 Trainium2 NeuronCores via Bass/Tile.

Strategy (matches the dst-sharded hint):
  - Nodes are range-partitioned across the 8 cores (6250 nodes/core).
  - Each core owns every edge whose dst lies in its node range, so the
    segment-sum over dst is fully core-local.
  - h (pre-scaled by outdeg^-0.5, cast to bf16) is replicated into each
    core's HBM at input staging time (this plays the role of the all-gather
    of src features); the per-edge random feat[src] read is an on-device
    dma_gather (custom SWDGE ucode) spread over 4 SWDGE queues.
  - The guaranteed self-loop edges (first N edges, src==dst) never touch
    the gather: their contribution alpha^d_v * feat[v] @ W is added in
    phase 2 as a second matmul (lhsT = host-staged, alpha-scaled own
    features transposed) accumulating into the same PSUM group.
  - Nodes are degree-ranked into windows (rank block w -> window w on all
    cores), so per-window tile counts are shared across cores (SPMD-safe)
    yet track the actual edge counts: early windows are tall, late windows
    short, with ~no padding. Within a window, each core's lo/hi count is
    padded to the 8-core max with index-0 entries so num_idxs_reg (the
    exact non-negative index count the ucode contract requires) is a
    shared compile-time constant; -1 entries beyond it generate no
    descriptors at all.
  - dma_gather indices are int16 (< 32768), so h is addressed as two
    tables (rows [0, 32768) and [32768, N)).
  - Windows are 64 dst nodes wide: the DVE sel-matrix build cost scales
    with window width, while the PE cost is LdWeights-bound and doesn't.

Device pipeline per core (all phase-1 operands bf16, PSUM fp32):
  phase 1 (edge aggregation, accumulates agg^T[feat, node] in SBUF):
    per 64-dst-node window: dma_gathers (lo/hi tables) fetch the window's
    h[src] rows; sel tiles are built up to 16-at-a-time with two wide DVE
    ops (is_equal against iota, then mult by alpha^dist), psum += matmul(
    lhsT=G_tile, rhs=sel_tile) accumulating over the window; the finished
    window column-block is copied (cast to bf16) into agg^T.
  phase 2: per 128-node block (2 windows): psum = matmul(lhsT=hsT block,
    rhs=W) + matmul(lhsT=agg^T block, rhs=W), then * s_v (per-partition,
    fp32) + bias into a staging SBUF tile; results leave in two large
    partition-major DMAs (mid-stream and final).
"""

import math

import numpy as np

P = 128
WIN = 64  # dst-window width (nodes per psum accumulation group)
ALPHA = 0.5
N_CORES = 8
SPLIT = 32768  # int16 index limit for dma_gather
KW = 16  # max sel tiles built per wide DVE op
GCH = 8  # max tiles per dma_gather (hw limit: <=1024 idxs/inst)


def _wrap_idx16(flat):
    """dma_gather index layout: entry k -> partition k%16, column k//16,
    replicated across the 8 gpsimd core groups (partitions 16-127)."""
    n = flat.shape[-1]
    assert n % 16 == 0
    cols = n // 16
    w = np.asarray(flat, np.int16).reshape(cols, 16).T  # [16, cols]
    return np.tile(w, (8, 1))  # [128, cols]


def _prep_host(src, dst, distance, n_cores, N):
    """Shard edges by dst; build per-core padded tile arrays.

    Returns per-window tile/index-count lists shared by all cores, plus the
    per-core gather/sel constants and the output inverse permutation.
    """
    E = src.shape[0]

    src = np.asarray(src).astype(np.int64)
    dst = np.asarray(dst).astype(np.int64)
    distance = np.asarray(distance).astype(np.int64)

    in_deg = np.bincount(dst, minlength=N).astype(np.float64)
    s_all = in_deg**-1.5  # applied after the W matmul

    # structural self-loops (first N edges): handled algebraically in
    # phase 2; excluded from the gather stream
    eidx = np.arange(E)
    self_mask = (eidx < N) & (src == dst)
    d_self = np.full(N, -1, np.int64)
    d_self[src[self_mask]] = distance[self_mask]

    gsrc = src[~self_mask]
    gdst = dst[~self_mask]
    gdist = distance[~self_mask]
    coef_all = np.float64(ALPHA) ** gdist.astype(np.float64)

    # Degree-ranked node -> (core, window, slot): rank block w (512 nodes)
    # becomes window w on every core; within a block, nodes are dealt
    # round-robin over cores in hi-degree order so the 8 per-core counts
    # stay within a few edges of each other.
    lo_deg = np.bincount(gdst[gsrc < SPLIT], minlength=N).astype(np.int64)
    hi_deg = np.bincount(gdst[gsrc >= SPLIT], minlength=N).astype(np.int64)
    order_nodes = np.argsort(-(lo_deg + hi_deg), kind="stable")
    blk = n_cores * WIN
    n_windows = (N + blk - 1) // blk
    node_core = np.empty(N, np.int64)
    node_window = np.empty(N, np.int64)
    node_slot = np.empty(N, np.int64)
    for w in range(n_windows):
        nodes_b = order_nodes[w * blk : (w + 1) * blk]
        nodes_b = nodes_b[np.argsort(-hi_deg[nodes_b], kind="stable")]
        pos = np.arange(nodes_b.shape[0])
        node_core[nodes_b] = pos % n_cores
        node_window[nodes_b] = w
        node_slot[nodes_b] = pos // n_cores

    core_of = node_core[gdst]
    w_of = node_window[gdst]
    r_of = node_slot[gdst].astype(np.float32)
    is_hi = (gsrc >= SPLIT).astype(np.int64)

    # per-(core, window, lo/hi) counts -> shared per-window capacities
    gw = (core_of * n_windows + w_of) * 2 + is_hi
    counts = np.bincount(gw, minlength=n_cores * n_windows * 2).reshape(
        n_cores, n_windows, 2
    )
    maxc = counts.max(axis=0)  # [n_windows, 2]
    reg_lo = (maxc[:, 0] + 15) // 16 * 16  # exact shared non-neg idx count
    reg_hi = (maxc[:, 1] + 15) // 16 * 16
    t_lo = np.maximum((reg_lo + P - 1) // P, 0)
    t_hi = np.maximum((reg_hi + P - 1) // P, 0)
    t_tot = t_lo + t_hi
    col_off = np.concatenate([[0], np.cumsum(t_tot)[:-1]])
    n_cols = int(t_tot.sum())

    # order edges by (core, window, lo/hi); q = position within group
    order = np.argsort(gw, kind="stable")
    sgw = gw[order]
    flat_counts = counts.reshape(-1)
    win_start = np.concatenate([[0], np.cumsum(flat_counts)[:-1]])
    q = np.arange(sgw.shape[0], dtype=np.int64) - win_start[sgw]

    core_arr = sgw // (2 * n_windows)
    w_arr = (sgw // 2) % n_windows
    hi_arr = sgw % 2
    j_arr = q // P + hi_arr * t_lo[w_arr]  # hi tiles after lo tiles
    p_arr = q % P
    col_arr = col_off[w_arr] + j_arr

    # r = -1 on pad slots (is_equal against iota 0..63 is then always 0)
    rofs = np.full((n_cores, P, n_cols), -1.0, np.float32)
    coef = np.zeros((n_cores, P, n_cols), np.float32)
    rofs[core_arr, p_arr, col_arr] = r_of[order]
    coef[core_arr, p_arr, col_arr] = coef_all[order].astype(np.float32)

    # gather indices, table-relative; -1 marks no-descriptor entries
    srcrel = np.full((n_cores, P, n_cols), -1, np.int64)
    srcrel[core_arr, p_arr, col_arr] = gsrc[order] - (gsrc[order] >= SPLIT) * SPLIT

    # pad every core's (window, lo/hi) group up to the shared reg count
    # with index 0 (a real descriptor; nullified by sel) so the shared
    # num_idxs_reg matches each core's non-negative count exactly
    for c in range(n_cores):
        for w in range(n_windows):
            for h2, reg in ((0, reg_lo[w]), (1, reg_hi[w])):
                cnt = counts[c, w, h2]
                j0 = col_off[w] + (t_lo[w] if h2 else 0)
                # flat positions [cnt, reg) within this group get idx 0
                if cnt < reg:
                    ppos = np.arange(cnt, reg)
                    srcrel[c, ppos % P, j0 + ppos // P] = 0

    # wrapped idx16 with per-window variable block sizes, 64B-aligned
    CLa = (t_lo * 8 + 31) // 32 * 32
    CHa = (t_hi * 8 + 31) // 32 * 32
    idx_off = np.concatenate([[0], np.cumsum(CLa + CHa)[:-1]])
    idx_cols = int((CLa + CHa).sum())
    idx16 = np.full((n_cores, P, idx_cols), -1, np.int16)
    for c in range(n_cores):
        flat = srcrel[c].T  # [n_cols, P]: (tile, lane)
        for w in range(n_windows):
            j0 = col_off[w]
            lo = flat[j0 : j0 + t_lo[w]].reshape(-1)
            hi = flat[j0 + t_lo[w] : j0 + t_tot[w]].reshape(-1)
            base = idx_off[w]
            if t_lo[w]:
                idx16[c, :, base : base + t_lo[w] * 8] = _wrap_idx16(lo)
            if t_hi[w]:
                idx16[c, :, base + CLa[w] : base + CLa[w] + t_hi[w] * 8] = (
                    _wrap_idx16(hi)
                )

    # per-node output scale, laid out for phase-2 blocks of 128 rows
    n_blocks = n_windows * WIN // P
    snode = np.ones((n_cores, P, n_blocks), np.float32)
    out_row = node_window * WIN + node_slot
    snode[node_core, out_row % P, out_row // P] = s_all.astype(np.float32)

    return (
        idx16, rofs, coef, snode, node_core, out_row, d_self,
        n_windows, t_lo, t_hi, reg_lo, reg_hi, col_off, CLa, CHa, idx_off,
        n_cols, idx_cols,
    )


def _build_nc(N, D, plan):
    import concourse.bacc as bacc
    import concourse.tile as tile
    from concourse import mybir

    (
        n_windows, t_lo, t_hi, reg_lo, reg_hi, col_off, CLa, CHa, idx_off,
        n_cols, idx_cols,
    ) = plan

    f32 = mybir.dt.float32
    bf16 = mybir.dt.bfloat16
    i16 = mybir.dt.int16
    n_blocks = n_windows * WIN // P  # phase-2 blocks of 128 output rows

    # bconst (bf16) free-dim layout: rofs | coef | iota | wmat
    btot = 2 * n_cols + WIN + D
    # fconst (f32) free-dim layout: biasf | snode
    ftot = D + n_blocks

    nc = bacc.Bacc(
        None,
        target_bir_lowering=False,
        debug=False,
        num_swdge_queues=4,
        dynamic_dma_scratch_size=98304,
    )
    h_d = nc.declare_dram_parameter("h", [N, D], bf16, isOutput=False)
    idx_d = nc.declare_dram_parameter("idx16", [P, idx_cols], i16, isOutput=False)
    bc_d = nc.declare_dram_parameter("bconst", [P, btot], bf16, isOutput=False)
    fc_d = nc.declare_dram_parameter("fconst", [P, ftot], f32, isOutput=False)
    # self-loop contribution: alpha^d_v * feat[v], transposed, block layout
    hs_d = nc.declare_dram_parameter("hself", [P, n_blocks * D], bf16, isOutput=False)
    # partition-major output: out[p, k*D:(k+1)*D] = block k, row p
    out_d = nc.declare_dram_parameter("out", [P, n_blocks * D], f32, isOutput=True)

    mult = mybir.AluOpType.mult

    with tile.TileContext(nc) as tc:
        with (
            tc.tile_pool(name="singles", bufs=1) as singles,
            tc.tile_pool(name="glo", bufs=4) as glopool,
            tc.tile_pool(name="ghi", bufs=4) as ghipool,
            tc.tile_pool(name="sel", bufs=4) as selpool,
            tc.tile_pool(name="psum", bufs=4, space="PSUM") as psumpool,
            tc.tile_pool(name="psum2", bufs=2, space="PSUM") as psum2pool,
        ):
            # idx chunks first on the sync HWDGE queue (they gate the first
            # gathers); big constants ride the Activation HWDGE queue so
            # they don't delay them
            idx_sb = singles.tile([P, idx_cols], i16)
            hd = min(idx_cols, max(256, int(idx_off[min(4, n_windows - 1)])))
            nc.sync.dma_start(out=idx_sb[:, :hd], in_=idx_d[:, :hd])
            if hd < idx_cols:
                nc.sync.dma_start(out=idx_sb[:, hd:], in_=idx_d[:, hd:])
            bc_sb = singles.tile([P, btot], bf16)
            nc.scalar.dma_start(out=bc_sb[:], in_=bc_d[:])
            fc_sb = singles.tile([P, ftot], f32)
            nc.scalar.dma_start(out=fc_sb[:], in_=fc_d[:])
            hs_sb = singles.tile([P, n_blocks * D], bf16)
            nc.scalar.dma_start(out=hs_sb[:], in_=hs_d[:])

            r_sb = bc_sb[:, 0:n_cols]
            c_sb = bc_sb[:, n_cols : 2 * n_cols]
            o0 = 2 * n_cols
            io_sb = bc_sb[:, o0 : o0 + WIN]
            w_sb = bc_sb[:, o0 + WIN : o0 + WIN + D]
            b_sb = fc_sb[:, 0:D]
            s_sb = fc_sb[:, D : D + n_blocks]

            agg = singles.tile([P, n_windows * WIN], bf16)  # agg^T [feat, node]
            o_all = singles.tile([P, n_blocks * D], f32)  # staged outputs

            def _phase2(k):
                ps2 = psum2pool.tile([P, D], f32)
                nc.tensor.matmul(
                    out=ps2[:],
                    lhsT=hs_sb[:, k * P : (k + 1) * P],
                    rhs=w_sb,
                    start=True,
                    stop=False,
                )
                nc.tensor.matmul(
                    out=ps2[:],
                    lhsT=agg[:, k * P : (k + 1) * P],
                    rhs=w_sb,
                    start=False,
                    stop=True,
                )
                o = o_all[:, k * D : (k + 1) * D]
                nc.vector.tensor_tensor(
                    out=o,
                    in0=ps2[:],
                    in1=s_sb[:, k : k + 1].to_broadcast([P, D]),
                    op=mult,
                )
                nc.vector.tensor_add(out=o, in0=o, in1=b_sb)

            h_lo = h_d[0 : min(SPLIT, N), :]
            hi_base = SPLIT if N > SPLIT else 0
            h_hi = h_d[hi_base:N, :]

            # Zero every physical gather buffer once: entries beyond
            # num_idxs_reg generate no descriptor, so whatever the buffer
            # holds leaks into the matmul lhsT. After this, stale contents
            # are always finite bf16 (earlier gathered rows), which sel's 0
            # column nullifies; raw uninitialized SBUF could decode as
            # NaN/Inf and NaN*0 would poison the accumulation.
            for _ in range(4):
                gz = glopool.tile([P, GCH, P], bf16, tag="glo")
                nc.vector.memset(gz[:], 0.0)
                gz = ghipool.tile([P, GCH, P], bf16, tag="ghi")
                nc.vector.memset(gz[:], 0.0)

            # phase-2 burst points (even window counts): draining output
            # work mid-stream keeps the tail after the last gather short
            mid1 = (n_windows // 2) // 2 * 2
            mid2 = (11 * n_windows // 12) // 2 * 2
            qctr = 0
            for w in range(n_windows):
                T_lo_w, T_hi_w = int(t_lo[w]), int(t_hi[w])
                T_w = T_lo_w + T_hi_w
                if T_w == 0:
                    nc.vector.memset(agg[:, w * WIN : (w + 1) * WIN], 0.0)
                    continue
                base = int(idx_off[w])
                chunks = []  # (tile_buf, ntiles) in tile order lo then hi
                for h2, T_t, reg, tbl, cb0, pool, tg in (
                    (0, T_lo_w, int(reg_lo[w]), h_lo, base, glopool, "glo"),
                    (1, T_hi_w, int(reg_hi[w]), h_hi, base + int(CLa[w]), ghipool, "ghi"),
                ):
                    for k in range((T_t + GCH - 1) // GCH):
                        nt = min(GCH, T_t - k * GCH)
                        creg = min(max(reg - k * GCH * P, 0), nt * P)
                        g = pool.tile([P, GCH, P], bf16, tag=tg)
                        cb = cb0 + k * GCH * 8
                        qn = qctr % 4
                        qctr += 1
                        nc.gpsimd.dma_gather(
                            g[:, :nt, :],
                            tbl,
                            idx_sb[:, cb : cb + nt * 8],
                            nt * P,
                            creg,
                            P,
                            single_packet=False,
                            queue_num=qn,
                        )
                        chunks.append((g, nt))

                # sel tiles for the whole window, built KW at a time with
                # two wide DVE ops (bf16 2x mode)
                sel_tiles = []
                for k0 in range(0, T_w, KW):
                    kk = min(KW, T_w - k0)
                    sw = selpool.tile([P, KW, WIN], bf16, tag="sel")
                    t0 = int(col_off[w]) + k0
                    r3 = r_sb[:, t0 : t0 + kk, None].to_broadcast([P, kk, WIN])
                    c3 = c_sb[:, t0 : t0 + kk, None].to_broadcast([P, kk, WIN])
                    io3 = io_sb[:, None, :].to_broadcast([P, kk, WIN])
                    nc.vector.tensor_tensor(
                        out=sw[:, :kk, :],
                        in0=r3,
                        in1=io3,
                        op=mybir.AluOpType.is_equal,
                    )
                    nc.vector.tensor_tensor(
                        out=sw[:, :kk, :], in0=sw[:, :kk, :], in1=c3, op=mult
                    )
                    sel_tiles.append(sw)

                ps = psumpool.tile([P, WIN], f32)
                j = 0
                for g, nt in chunks:
                    for jj in range(nt):
                        nc.tensor.matmul(
                            out=ps[:],
                            lhsT=g[:, jj, :],
                            rhs=sel_tiles[j // KW][:, j % KW, :],
                            start=(j == 0),
                            stop=(j == T_w - 1),
                        )
                        j += 1
                nc.scalar.copy(out=agg[:, w * WIN : (w + 1) * WIN], in_=ps[:])

                if w in (mid1 - 1, mid2 - 1):
                    # mid-stream burst: finish output for the windows already
                    # aggregated, while gathers for the rest continue
                    k0 = 0 if w == mid1 - 1 else mid1 * WIN // P
                    k1 = (w + 1) * WIN // P
                    for k in range(k0, k1):
                        _phase2(k)
                    nc.scalar.dma_start(
                        out=out_d[:, k0 * D : k1 * D],
                        in_=o_all[:, k0 * D : k1 * D],
                    )
            k2 = mid2 * WIN // P
            for k in range(k2, n_blocks):
                _phase2(k)
            nc.scalar.dma_start(out=out_d[:, k2 * D :], in_=o_all[:, k2 * D :])

    nc.compile()
    return nc


def kernel(h, src, dst, distance, weight, bias, _trace=False):
    import ml_dtypes
    from concourse.bass_utils import run_bass_kernel_spmd

    h = np.asarray(h, dtype=np.float32)
    weight = np.asarray(weight, dtype=np.float32)
    bias = np.asarray(bias, dtype=np.float32)
    N, D = h.shape

    (
        idx16, rofs, coef, snode, node_core, out_row, d_self,
        n_windows, t_lo, t_hi, reg_lo, reg_hi, col_off, CLa, CHa, idx_off,
        n_cols, idx_cols,
    ) = _prep_host(src, dst, distance, N_CORES, N)

    # source-side norm folded into the (bf16) feature table
    out_deg = np.bincount(np.asarray(src).astype(np.int64), minlength=N)
    h_pre = h * (out_deg.astype(np.float64) ** -0.5)[:, None].astype(np.float32)
    h_bf = np.ascontiguousarray(h_pre.astype(ml_dtypes.bfloat16))

    # self-loop table: hsT[:, out_row[v]] = alpha^d_self[v] * feat[v]
    n_blocks = n_windows * WIN // P
    hsT = np.zeros((N_CORES, D, n_blocks * P), np.float32)
    has_self = d_self >= 0
    vs = np.nonzero(has_self)[0]
    hsT[node_core[vs], :, out_row[vs]] = (
        h_pre[vs] * (np.float32(ALPHA) ** d_self[vs].astype(np.float32))[:, None]
    )

    iota = np.broadcast_to(np.arange(WIN, dtype=np.float32)[None, :], (P, WIN))
    biasf = np.broadcast_to(bias[None, :], (P, D))

    plan = (
        n_windows, t_lo, t_hi, reg_lo, reg_hi, col_off, CLa, CHa, idx_off,
        n_cols, idx_cols,
    )
    nc = _build_nc(N, D, plan)

    in_maps = []
    for c in range(N_CORES):
        bconst = np.concatenate([rofs[c], coef[c], iota, weight], axis=1).astype(
            ml_dtypes.bfloat16
        )
        fconst = np.concatenate([biasf, snode[c]], axis=1).astype(np.float32)
        in_maps.append(
            {
                "h": h_bf,
                "idx16": np.ascontiguousarray(idx16[c]),
                "bconst": np.ascontiguousarray(bconst),
                "fconst": np.ascontiguousarray(fconst),
                "hself": np.ascontiguousarray(hsT[c].astype(ml_dtypes.bfloat16)),
            }
        )

    res = run_bass_kernel_spmd(nc, in_maps, list(range(N_CORES)), trace=_trace)

    # out[p, k*D:(k+1)*D] holds node (core, row k*128+p); un-permute
    stacked = np.stack([res.results[c]["out"] for c in range(N_CORES)])
    per_core = stacked.reshape(N_CORES, P, n_blocks, D).transpose(0, 2, 1, 3)
    out = per_core[node_core, out_row // P, out_row % P].astype(np.float32)

    if _trace:
        return out, res
    return out
